# revision 16
# baseline (speedup 1.0000x reference)
"""CTConv2d Trainium2 kernel.

Computes y = conv2d(x, w) where w (O,I,3,3) is synthesized on host from
core/periphery/threshold/scale (tiny tensors), and the conv runs on 8
NeuronCores, data-parallel over batch (32 images -> 4 per core).

Device kernel (per core): the synthesized weight is
    w[o,i,dh,dw] = c[o,i] (center) / c*gate (cg) * p[dh,dw] (periphery),
so per 4-row output block the PE runs 5 matmuls (N=448 into fp32 PSUM):
  - 1 fp16 matmul for the center tap (accuracy-critical, ~93% of the
    output energy), moving operand = the raw fp16 image.
  - 4 fp8e4 DoubleRow matmuls, each covering a tap PAIR in one pass
    (DoubleRow packs 2 fp8 weights per PE cell, virtual K=256):
    top corners (dh=-1, dw=+-1), bottom corners (dh=+1, dw=+-1),
    vertical pair (dh=+-1, dw=0), horizontal pair (dh=0, dw=+-1).
    The moving operand is a hand-built 4D access pattern over the
    zero-padded fp8 image whose dim-1 is the pair: stride 2 elements
    for the +-dw pairs, 2 rows for the +-dh pair.
All weights are pre-scaled by S=1024 so the tiny fp8 weights (~4e-3)
land in e4m3's normal range; the grouped PSUM->SBUF copy on the scalar
engine compensates with its free affine scale (1/S) and emits fp16,
halving the output DMA. Per-image input: the raw fp16 image (center)
plus a zero-padded fp8e4 image (stride 120) for the DoubleRow taps,
chunk-loaded with the next image's chunks interleaved into the block
loop so prefetch never competes with the current image's tail chunks.
PSUM: 2 four-bank tiles, 4 blocks per grouped copy; the last image
ends with two 2-block groups on alternating tiles (a copy that
interleaves with matmuls into the same PSUM tile serializes on the
tile-granular WAR dependency). Accumulation is fp32; absmax rel err
~5e-3 vs the fp32 reference. ~126-130us on 8 cores (baseline 174us
fp16 7-tap, PE-streaming floor for this decomposition ~124us).
"""

import os
import sys

# The grading/bench environment may pin JAX_PLATFORMS=cpu for the jax
# reference; this kernel needs the axon/neuron PJRT backend.
if os.environ.get("JAX_PLATFORMS") == "cpu":
    del os.environ["JAX_PLATFORMS"]

for _p in ("/opt/trn_rl_repo",):
    if os.path.isdir(_p) and _p not in sys.path:
        sys.path.append(_p)

import numpy as np
import ml_dtypes
import bass_rust

import concourse.mybir as mybir
from concourse import bacc
from concourse.bass_utils import run_bass_kernel_spmd
from concourse.tile import TileContext

O = 128
I = 128
B = 32
H = 112
W = 112
NCORES = 8
BPC = B // NCORES  # images per core
HP = H + 2  # fp8 padded rows (interior at row 1)
WP8 = 120  # fp8 image row stride (2*WP8 % 16 == 0 for the DR pair step)
C8 = 4  # fp8 interior column offset
RB = 4  # output rows per block (N = RB*W = 448 <= 512)
NBLK = H // RB  # 28
GRP = 4  # blocks per PSUM tile / grouped copy / output DMA
NG = NBLK // GRP  # 7 groups per image
SCL = 1024.0  # weight pre-scale so fp8 weights sit in e4m3 normal range
BANK = 512  # PSUM bank stride in fp32 elements
# image-load chunks (row ranges); first chunk small so block 0 starts asap
CHUNKS8 = [(0, 6), (6, 14), (14, 26), (26, 42), (42, 58), (58, 74), (74, 94), (94, HP)]
CHUNKS16 = [(0, 5), (5, 13), (13, 25), (25, 41), (41, 57), (57, 73), (73, 93), (93, H)]
# image-b block index at which chunk ci of image b+1 is issued
PREFETCH_AT = [3, 6, 9, 12, 15, 18, 21, 24]
F32 = mybir.dt.float32
F16 = mybir.dt.float16
F8 = mybir.dt.float8e4

# DoubleRow tap pairs: (name, pair-dim kind, padded-row offset rel. to h0,
# col offset rel. to C8): pair stride 2 elems for dw=+-1, 2 rows for dh=+-1
# slot0/slot1 weights are p[dh,dw] for the two paired taps.
DR_TAPS = [
    ((-1, -1), (-1, +1), "col", 0),  # top corners: row h0+1-1 = h0
    ((+1, -1), (+1, +1), "col", 2),  # bottom corners: row h0+2
    ((-1, 0), (+1, 0), "row", 0),  # vertical pair: rows h0, h0+2
    ((0, -1), (0, +1), "col", 1),  # horizontal pair: row h0+1
]


def synth_weights(core, periphery, threshold, scale):
    """Host-side weight synthesis.

    Returns (w16, w8):
      w16 (I, O) fp16 lhsT: center tap c*S.
      w8 (I, 4*2*O) fp8e4 lhsT, per DR_TAPS block t:
        [t*2*O + slot*O + o] = cg * p[tap_slot] * S.
    """
    c = np.asarray(core, np.float64)[:, :, 0, 0]  # (O, I)
    thr = np.asarray(threshold, np.float64)
    s = float(np.asarray(scale, np.float64)[0])
    p = np.asarray(periphery, np.float64)
    gate = 1.0 / (1.0 + np.exp(-s * (np.abs(c) - thr[:, None])))
    p_full = np.concatenate([p[:4], [1.0], p[4:]])  # (9,) taps row-major
    cg = c * gate

    def ptap(dh, dw):
        return p_full[(dh + 1) * 3 + (dw + 1)]

    w16 = np.ascontiguousarray((c * SCL).T.astype(np.float16))

    w8 = np.empty((4, 2, O, I), np.float64)
    for t, (tapA, tapB, _, _) in enumerate(DR_TAPS):
        w8[t, 0] = cg * ptap(*tapA) * SCL
        w8[t, 1] = cg * ptap(*tapB) * SCL
    w8 = w8.transpose(3, 0, 1, 2).reshape(I, 4 * 2 * O)
    w8 = np.clip(w8, -240.0, 240.0)
    return w16, np.ascontiguousarray(w8.astype(ml_dtypes.float8_e4m3))


def _dr_rhs(img8v, h0, kind, roff):
    """Moving-operand AP for one DoubleRow pair MM at block row h0.

    kind='col': pair = cols (C8-1, C8+1) at padded row h0+roff.
    kind='row': pair = padded rows (h0, h0+2) at col C8.
    Free dims [pair:2, h:RB, w:W] -> free size 2*RB*W = 896.
    """
    if kind == "col":
        sl = img8v[:, h0 + roff : h0 + roff + RB, C8 - 1 : C8 + 1 + W]
        pair = [2, 2]
    else:
        sl = img8v[:, h0 : h0 + RB + 2, C8 : C8 + W]
        pair = [2 * WP8, 2]
    rhs = sl.copy()
    rhs.ap = bass_rust.VecI64Pair([list(sl.ap[0]), pair, [WP8, RB], [1, W]])
    return rhs


def build_nc():
    nc = bacc.Bacc(None)
    x16_d = nc.dram_tensor("x16", [BPC, I, H * W], F16, kind="ExternalInput")
    x8_d = nc.dram_tensor("x8", [BPC, I, HP * WP8], F8, kind="ExternalInput")
    w16_d = nc.dram_tensor("w16", [I, O], F16, kind="ExternalInput")
    w8_d = nc.dram_tensor("w8", [I, 4 * 2 * O], F8, kind="ExternalInput")
    y_d = nc.dram_tensor("y", [BPC, O, H, W], F16, kind="ExternalOutput")

    with TileContext(nc) as tc, tc.tile_pool(name="persist", bufs=1) as persist:
        w16t = persist.tile([I, O], F16, name="w16t", tag="w16t")
        w8t = persist.tile([I, 4 * 2 * O], F8, name="w8t", tag="w8t")
        # weights on the ACT ring (idle at start; the out DMAs come later),
        # image chunks on the SP ring -> the two loads run in parallel.
        nc.scalar.dma_start(out=w16t[:], in_=w16_d[:])
        nc.scalar.dma_start(out=w8t[:], in_=w8_d[:])

        imgs16 = []
        imgs8 = []
        for ib in range(2):
            imgs16.append(
                persist.tile([128, H * W], F16, name=f"i16_{ib}", tag=f"i16_{ib}")
            )
            imgs8.append(
                persist.tile([128, HP * WP8], F8, name=f"i8_{ib}", tag=f"i8_{ib}")
            )

        # HAM warmup: the PE clock gate sits at 1.2 GHz until ~3.4us of
        # sustained matmul activity. A dependency-free burst right after
        # engine boot flips it to 2.4 GHz before the first real matmul
        # (which waits on the weight/first-chunk DMA receipt chain anyway).
        warm = persist.tile([128, 640], F16, name="warm", tag="warm")
        nc.vector.memset(warm[:], 0.0)

        def load_chunk(b, ci):
            i16 = imgs16[b % 2]
            i8 = imgs8[b % 2]
            r0, r1 = CHUNKS16[ci]
            nc.sync.dma_start(
                out=i16[:, r0 * W : r1 * W], in_=x16_d[b][:, r0 * W : r1 * W]
            )
            r0, r1 = CHUNKS8[ci]
            nc.sync.dma_start(
                out=i8[:, r0 * WP8 : r1 * WP8], in_=x8_d[b][:, r0 * WP8 : r1 * WP8]
            )

        with (
            tc.tile_pool(name="psum", bufs=2, space="PSUM") as psum_pool,
            tc.tile_pool(name="outp", bufs=3) as out_pool,
        ):
            for ci in range(len(CHUNKS8)):
                load_chunk(0, ci)
            for k in range(10):
                pw = psum_pool.tile([128, GRP * BANK], F32, name="pw", tag="ps")
                nc.tensor.matmul(
                    out=pw[:, 0:512],
                    lhsT=warm[:, 0:128],
                    rhs=warm[:, 128:640],
                    start=True,
                    stop=True,
                )
            for b in range(BPC):
                img16v = imgs16[b % 2].rearrange("p (h w) -> p h w", w=W)
                img8v = imgs8[b % 2].rearrange("p (h w) -> p h w", w=WP8)
                yflat = y_d[b].rearrange("o h w -> o (h w)")
                # group plan: the last image ends with two 2-block groups on
                # alternating PSUM tiles, so the final copies never interleave
                # with matmuls into the same tile (intra-tile WAR serializes)
                # and the post-last-matmul tail is one short copy + small DMA.
                plan = [GRP] * NG if b < BPC - 1 else [GRP] * (NG - 1) + [2, 1, 1]
                g_of = {}
                blk0 = 0
                for gsz in plan:
                    for j in range(gsz):
                        g_of[blk0 + j] = (blk0, gsz)
                    blk0 += gsz
                ps = None
                ot = None
                for blk in range(NBLK):
                    if b + 1 < BPC and blk in PREFETCH_AT:
                        load_chunk(b + 1, PREFETCH_AT.index(blk))
                    g0, gsz = g_of[blk]
                    j = blk - g0
                    if j == 0:
                        ps = psum_pool.tile([128, GRP * BANK], F32, name="ps")
                        ot = out_pool.tile([128, gsz * RB * W], F16, name="ot")
                    h0 = blk * RB
                    pslice = ps[:, j * BANK : j * BANK + RB * W]
                    nc.tensor.matmul(
                        out=pslice,
                        lhsT=w16t[:],
                        rhs=img16v[:, h0 : h0 + RB, 0:W],
                        start=True,
                        stop=False,
                    )
                    for t, (_, _, kind, roff) in enumerate(DR_TAPS):
                        nc.tensor.matmul(
                            out=pslice,
                            lhsT=w8t[:, t * 2 * O : (t + 1) * 2 * O].rearrange(
                                "p (pair o) -> p pair o", pair=2
                            ),
                            rhs=_dr_rhs(img8v, h0, kind, roff),
                            start=False,
                            stop=(t == len(DR_TAPS) - 1),
                            perf_mode=mybir.MatmulPerfMode.DoubleRow,
                        )
                    if j == gsz - 1:
                        # grouped PSUM->SBUF copy: one ACTIVATE over the
                        # group's banks, free affine scale undoing SCL,
                        # fp16 out (halves output DMA traffic).
                        n = RB * W
                        ps4 = ps.rearrange("p (g c) -> p g c", c=BANK)
                        ot3 = ot.rearrange("p (g c) -> p g c", c=n)
                        nc.scalar.mul(
                            out=ot3[:], in_=ps4[:, 0:gsz, 0:n], mul=1.0 / SCL
                        )
                        nc.scalar.dma_start(
                            out=yflat[:, g0 * n : (g0 + gsz) * n], in_=ot[:]
                        )
    nc.finalize()
    return nc


_NC_CACHE = {}


def _get_nc():
    if "nc" not in _NC_CACHE:
        _NC_CACHE["nc"] = build_nc()
    return _NC_CACHE["nc"]


def _prep_images(x):
    """(B, I, H, W) fp32 -> raw fp16 image and zero-padded fp8e4 image."""
    x16 = np.ascontiguousarray(x.astype(np.float16).reshape(B, I, H * W))
    xp8 = np.zeros((B, I, HP, WP8), ml_dtypes.float8_e4m3)
    xp8[:, :, 1 : 1 + H, C8 : C8 + W] = x.astype(ml_dtypes.float8_e4m3)
    return x16, np.ascontiguousarray(xp8.reshape(B, I, HP * WP8))


def run(inputs, trace=False, **kw):
    """Run on hardware; returns (y, BassKernelResults)."""
    x = np.asarray(inputs["x"], np.float32)
    assert x.shape == (B, I, H, W), x.shape
    w16, w8 = synth_weights(
        inputs["core"], inputs["periphery"], inputs["threshold"], inputs["scale"]
    )
    x16, x8 = _prep_images(x)
    nc = _get_nc()
    in_maps = [
        {
            "x16": x16[c * BPC : (c + 1) * BPC],
            "x8": x8[c * BPC : (c + 1) * BPC],
            "w16": w16,
            "w8": w8,
        }
        for c in range(NCORES)
    ]
    res = run_bass_kernel_spmd(nc, in_maps, list(range(NCORES)), trace=trace, **kw)
    y = np.concatenate(
        [res.results[c]["y"].astype(np.float32) for c in range(NCORES)], axis=0
    )
    return y, res


def kernel(**inputs) -> np.ndarray:
    y, _ = run(inputs)
    return y


# revision 17
# speedup vs baseline: 1.1179x; 1.1179x over previous
"""CTConv2d Trainium2 kernel.

Computes y = conv2d(x, w) where w (O,I,3,3) is synthesized on host from
core/periphery/threshold/scale (tiny tensors), and the conv runs on 8
NeuronCores, data-parallel over batch (32 images -> 4 per core).

Device kernel (per core): the synthesized weight is
    w[o,i,dh,dw] = c[o,i] (center) / c*gate (cg) * p[dh,dw] (periphery),
so per 4-row output block the PE runs 5 matmuls (N=448 into fp32 PSUM):
  - 1 fp16 matmul for the center tap (accuracy-critical, ~93% of the
    output energy), moving operand = the raw fp16 image.
  - 4 fp8e4 DoubleRow matmuls, each covering a tap PAIR in one pass
    (DoubleRow packs 2 fp8 weights per PE cell, virtual K=256):
    top corners (dh=-1, dw=+-1), bottom corners (dh=+1, dw=+-1),
    vertical pair (dh=+-1, dw=0), horizontal pair (dh=0, dw=+-1).
    The moving operand is a hand-built 4D access pattern over the
    zero-padded fp8 image whose dim-1 is the pair: stride 2 elements
    for the +-dw pairs, 2 rows for the +-dh pair.
All weights are pre-scaled by S=1024 so the tiny fp8 weights (~4e-3)
land in e4m3's normal range; the grouped PSUM->SBUF copy on the scalar
engine compensates with its free affine scale (1/S) and emits fp16,
halving the output DMA. Per-image input: the raw fp16 image (center)
plus a zero-padded fp8e4 image (stride 120) for the DoubleRow taps,
chunk-loaded with the next image's chunks interleaved into the block
loop so prefetch never competes with the current image's tail chunks.
PSUM: 2 four-bank tiles, 4 blocks per grouped copy; the last image
ends with two 2-block groups on alternating tiles (a copy that
interleaves with matmuls into the same PSUM tile serializes on the
tile-granular WAR dependency). Accumulation is fp32; absmax rel err
~5e-3 vs the fp32 reference. ~126-130us on 8 cores (baseline 174us
fp16 7-tap, PE-streaming floor for this decomposition ~124us).
"""

import os
import sys

# The grading/bench environment may pin JAX_PLATFORMS=cpu for the jax
# reference; this kernel needs the axon/neuron PJRT backend.
if os.environ.get("JAX_PLATFORMS") == "cpu":
    del os.environ["JAX_PLATFORMS"]

for _p in ("/opt/trn_rl_repo",):
    if os.path.isdir(_p) and _p not in sys.path:
        sys.path.append(_p)

import numpy as np
import ml_dtypes
import bass_rust

import concourse.mybir as mybir
from concourse import bacc
from concourse.bass_utils import run_bass_kernel_spmd
from concourse.tile import TileContext

O = 128
I = 128
B = 32
H = 112
W = 112
NCORES = 8
BPC = B // NCORES  # images per core
HP = H + 2  # fp8 padded rows (interior at row 1)
WP8 = 120  # fp8 image row stride (2*WP8 % 16 == 0 for the DR pair step)
C8 = 4  # fp8 interior column offset
RB = 4  # output rows per block (N = RB*W = 448 <= 512)
NBLK = H // RB  # 28
GRP = 4  # blocks per PSUM tile / grouped copy / output DMA
NG = NBLK // GRP  # 7 groups per image
SCL = 1024.0  # weight pre-scale so fp8 weights sit in e4m3 normal range
BANK = 512  # PSUM bank stride in fp32 elements
# image-load chunks (row ranges); first chunk small so block 0 starts asap
CHUNKS8 = [(0, 6), (6, 14), (14, 26), (26, 42), (42, 58), (58, 74), (74, 94), (94, HP)]
CHUNKS16 = [(0, 5), (5, 13), (13, 25), (25, 41), (41, 57), (57, 73), (73, 93), (93, H)]
# image-b block index at which chunk ci of image b+1 is issued
PREFETCH_AT = [3, 6, 9, 12, 15, 18, 21, 24]
F32 = mybir.dt.float32
F16 = mybir.dt.float16
F8 = mybir.dt.float8e4

# DoubleRow tap pairs: (name, pair-dim kind, padded-row offset rel. to h0,
# col offset rel. to C8): pair stride 2 elems for dw=+-1, 2 rows for dh=+-1
# slot0/slot1 weights are p[dh,dw] for the two paired taps.
DR_TAPS = [
    ((-1, -1), (-1, +1), "col", 0),  # top corners: row h0+1-1 = h0
    ((+1, -1), (+1, +1), "col", 2),  # bottom corners: row h0+2
    ((-1, 0), (+1, 0), "row", 0),  # vertical pair: rows h0, h0+2
    ((0, -1), (0, +1), "col", 1),  # horizontal pair: row h0+1
]


def synth_weights(core, periphery, threshold, scale):
    """Host-side weight synthesis.

    Returns (w16, w8):
      w16 (I, O) fp16 lhsT: center tap c*S.
      w8 (I, 4*2*O) fp8e4 lhsT, per DR_TAPS block t:
        [t*2*O + slot*O + o] = cg * p[tap_slot] * S.
    """
    c = np.asarray(core, np.float64)[:, :, 0, 0]  # (O, I)
    thr = np.asarray(threshold, np.float64)
    s = float(np.asarray(scale, np.float64)[0])
    p = np.asarray(periphery, np.float64)
    gate = 1.0 / (1.0 + np.exp(-s * (np.abs(c) - thr[:, None])))
    p_full = np.concatenate([p[:4], [1.0], p[4:]])  # (9,) taps row-major
    cg = c * gate

    def ptap(dh, dw):
        return p_full[(dh + 1) * 3 + (dw + 1)]

    w16 = np.ascontiguousarray((c * SCL).T.astype(np.float16))

    w8 = np.empty((4, 2, O, I), np.float64)
    for t, (tapA, tapB, _, _) in enumerate(DR_TAPS):
        w8[t, 0] = cg * ptap(*tapA) * SCL
        w8[t, 1] = cg * ptap(*tapB) * SCL
    w8 = w8.transpose(3, 0, 1, 2).reshape(I, 4 * 2 * O)
    w8 = np.clip(w8, -240.0, 240.0)
    return w16, np.ascontiguousarray(w8.astype(ml_dtypes.float8_e4m3))


def _dr_rhs(img8v, h0, kind, roff):
    """Moving-operand AP for one DoubleRow pair MM at block row h0.

    kind='col': pair = cols (C8-1, C8+1) at padded row h0+roff.
    kind='row': pair = padded rows (h0, h0+2) at col C8.
    Free dims [pair:2, h:RB, w:W] -> free size 2*RB*W = 896.
    """
    if kind == "col":
        sl = img8v[:, h0 + roff : h0 + roff + RB, C8 - 1 : C8 + 1 + W]
        pair = [2, 2]
    else:
        sl = img8v[:, h0 : h0 + RB + 2, C8 : C8 + W]
        pair = [2 * WP8, 2]
    rhs = sl.copy()
    rhs.ap = bass_rust.VecI64Pair([list(sl.ap[0]), pair, [WP8, RB], [1, W]])
    return rhs


def build_nc():
    nc = bacc.Bacc(None)
    x16_d = nc.dram_tensor("x16", [BPC, I, H * W], F16, kind="ExternalInput")
    x8_d = nc.dram_tensor("x8", [BPC, I, HP * WP8], F8, kind="ExternalInput")
    w16_d = nc.dram_tensor("w16", [I, O], F16, kind="ExternalInput")
    w8_d = nc.dram_tensor("w8", [I, 4 * 2 * O], F8, kind="ExternalInput")
    y_d = nc.dram_tensor("y", [BPC, O, H, W], F16, kind="ExternalOutput")

    with TileContext(nc) as tc, tc.tile_pool(name="persist", bufs=1) as persist:
        w16t = persist.tile([I, O], F16, name="w16t", tag="w16t")
        w8t = persist.tile([I, 4 * 2 * O], F8, name="w8t", tag="w8t")
        # weights on the ACT ring (idle at start; the out DMAs come later),
        # image chunks on the SP ring -> the two loads run in parallel.
        nc.scalar.dma_start(out=w16t[:], in_=w16_d[:])
        nc.scalar.dma_start(out=w8t[:], in_=w8_d[:])

        imgs16 = []
        imgs8 = []
        for ib in range(2):
            imgs16.append(
                persist.tile([128, H * W], F16, name=f"i16_{ib}", tag=f"i16_{ib}")
            )
            imgs8.append(
                persist.tile([128, HP * WP8], F8, name=f"i8_{ib}", tag=f"i8_{ib}")
            )

        # HAM warmup: the PE clock gate sits at 1.2 GHz until ~3.4us of
        # sustained matmul activity. A dependency-free burst right after
        # engine boot flips it to 2.4 GHz before the first real matmul
        # (which waits on the weight/first-chunk DMA receipt chain anyway).
        warm = persist.tile([128, 640], F16, name="warm", tag="warm")
        nc.vector.memset(warm[:], 0.0)

        def load_chunk(b, ci):
            i16 = imgs16[b % 2]
            i8 = imgs8[b % 2]
            r0, r1 = CHUNKS16[ci]
            nc.sync.dma_start(
                out=i16[:, r0 * W : r1 * W], in_=x16_d[b][:, r0 * W : r1 * W]
            )
            r0, r1 = CHUNKS8[ci]
            nc.sync.dma_start(
                out=i8[:, r0 * WP8 : r1 * WP8], in_=x8_d[b][:, r0 * WP8 : r1 * WP8]
            )

        with (
            tc.tile_pool(name="psum", bufs=2, space="PSUM") as psum_pool,
            tc.tile_pool(name="outp", bufs=3) as out_pool,
        ):
            for ci in range(len(CHUNKS8)):
                load_chunk(0, ci)
            for k in range(10):
                pw = psum_pool.tile([128, GRP * BANK], F32, name="pw", tag="ps")
                nc.tensor.matmul(
                    out=pw[:, 0:512],
                    lhsT=warm[:, 0:128],
                    rhs=warm[:, 128:640],
                    start=True,
                    stop=True,
                )
            for b in range(BPC):
                img16v = imgs16[b % 2].rearrange("p (h w) -> p h w", w=W)
                img8v = imgs8[b % 2].rearrange("p (h w) -> p h w", w=WP8)
                yflat = y_d[b].rearrange("o h w -> o (h w)")
                # group plan: the last image ends with two 2-block groups on
                # alternating PSUM tiles, so the final copies never interleave
                # with matmuls into the same tile (intra-tile WAR serializes)
                # and the post-last-matmul tail is one short copy + small DMA.
                plan = [GRP] * NG if b < BPC - 1 else [GRP] * (NG - 1) + [2, 2]
                g_of = {}
                blk0 = 0
                for gsz in plan:
                    for j in range(gsz):
                        g_of[blk0 + j] = (blk0, gsz)
                    blk0 += gsz
                ps = None
                ot = None
                for blk in range(NBLK):
                    if b + 1 < BPC and blk in PREFETCH_AT:
                        load_chunk(b + 1, PREFETCH_AT.index(blk))
                    g0, gsz = g_of[blk]
                    j = blk - g0
                    if j == 0:
                        ps = psum_pool.tile([128, GRP * BANK], F32, name="ps")
                        ot = out_pool.tile([128, gsz * RB * W], F16, name="ot")
                    h0 = blk * RB
                    pslice = ps[:, j * BANK : j * BANK + RB * W]
                    nc.tensor.matmul(
                        out=pslice,
                        lhsT=w16t[:],
                        rhs=img16v[:, h0 : h0 + RB, 0:W],
                        start=True,
                        stop=False,
                    )
                    for t, (_, _, kind, roff) in enumerate(DR_TAPS):
                        nc.tensor.matmul(
                            out=pslice,
                            lhsT=w8t[:, t * 2 * O : (t + 1) * 2 * O].rearrange(
                                "p (pair o) -> p pair o", pair=2
                            ),
                            rhs=_dr_rhs(img8v, h0, kind, roff),
                            start=False,
                            stop=(t == len(DR_TAPS) - 1),
                            perf_mode=mybir.MatmulPerfMode.DoubleRow,
                        )
                    if j == gsz - 1:
                        # grouped PSUM->SBUF copy: one ACTIVATE over the
                        # group's banks, free affine scale undoing SCL,
                        # fp16 out (halves output DMA traffic).
                        n = RB * W
                        ps4 = ps.rearrange("p (g c) -> p g c", c=BANK)
                        ot3 = ot.rearrange("p (g c) -> p g c", c=n)
                        nc.scalar.mul(
                            out=ot3[:], in_=ps4[:, 0:gsz, 0:n], mul=1.0 / SCL
                        )
                        nc.scalar.dma_start(
                            out=yflat[:, g0 * n : (g0 + gsz) * n], in_=ot[:]
                        )
    nc.finalize()
    return nc


_NC_CACHE = {}


def _get_nc():
    if "nc" not in _NC_CACHE:
        _NC_CACHE["nc"] = build_nc()
    return _NC_CACHE["nc"]


def _prep_images(x):
    """(B, I, H, W) fp32 -> raw fp16 image and zero-padded fp8e4 image."""
    x16 = np.ascontiguousarray(x.astype(np.float16).reshape(B, I, H * W))
    xp8 = np.zeros((B, I, HP, WP8), ml_dtypes.float8_e4m3)
    xp8[:, :, 1 : 1 + H, C8 : C8 + W] = x.astype(ml_dtypes.float8_e4m3)
    return x16, np.ascontiguousarray(xp8.reshape(B, I, HP * WP8))


def run(inputs, trace=False, **kw):
    """Run on hardware; returns (y, BassKernelResults)."""
    x = np.asarray(inputs["x"], np.float32)
    assert x.shape == (B, I, H, W), x.shape
    w16, w8 = synth_weights(
        inputs["core"], inputs["periphery"], inputs["threshold"], inputs["scale"]
    )
    x16, x8 = _prep_images(x)
    nc = _get_nc()
    in_maps = [
        {
            "x16": x16[c * BPC : (c + 1) * BPC],
            "x8": x8[c * BPC : (c + 1) * BPC],
            "w16": w16,
            "w8": w8,
        }
        for c in range(NCORES)
    ]
    res = run_bass_kernel_spmd(nc, in_maps, list(range(NCORES)), trace=trace, **kw)
    y = np.concatenate(
        [res.results[c]["y"].astype(np.float32) for c in range(NCORES)], axis=0
    )
    return y, res


def kernel(**inputs) -> np.ndarray:
    y, _ = run(inputs)
    return y
